# revision 10
# baseline (speedup 1.0000x reference)
"""Sliding-window attention (WINDOW=129) Trainium2 Bass kernel.

Problem: x[B=2, N=2048, C=768] -> qkv proj -> 12-head sliding-window
attention (half-window 64) -> output proj + bias.

Sharding: sequence-parallel over 8 cores: core c handles batch b = c//4,
query chunk s = c%4 (512 queries), with a 64-row halo each side for K/V.
Weights replicated. Each core computes its 512 output rows completely;
host concatenates. No collectives.

Per-core pipeline (matmul operands fp16, psum f32):
  qkT [e, n] via M=128 matmuls; each psum pair tile (two 64-row head
    groups stacked) is evacuated with ONE f32->f16 copy keeping the odd
    group at partition base 64 -- scores matmuls read it there directly
    via tile_position=(64, 0) (PE quadrant addressing; probed on HW).
  v -> vaug [n, 12*(64+1)] with a fused ones column per head.
  scores sT[k, q] per key-tile via K=64 matmuls; exp on ACT (scores are
    N(0,1)-scale so no max subtraction); 0/1 band/validity mask TT split
    across DVE/Pool.
  AV with q on the output partition dim: out[q, 65] = pT_slice.T @ vaug;
    column 64 gives the softmax denominator; reciprocal + per-partition
    broadcast multiply normalizes; per-128-col DMA transposes (XBAR)
    produce attnT [c, n] off the PE critical path.
  proj matmul + bias add (Pool) -> f16 out DMA.
Startup DMAs interleave the critical qkT inputs (wqk q-halves + all xT
tiles) round-robin across the three DMA-capable queues; the k-halves
and later-phase weights stream on sync behind them.
"""

import numpy as np

import concourse.bass as bass
import concourse.tile as tile
from concourse import bacc, mybir
from concourse._compat import with_exitstack
from concourse.masks import make_identity

B, N, C = 2, 2048, 768
H, D = 12, 64
HALF = 64            # half window
NCORES = 8
CHUNK = 512          # queries per core
NK = CHUNK + 2 * HALF  # 640 rows incl halo
SCALE = D ** -0.5

F16 = mybir.dt.float16
F32 = mybir.dt.float32
F8 = mybir.dt.float8e4
DR = mybir.MatmulPerfMode.DoubleRow
XS = 2.0 ** 2       # fp8 pre-scale on x
WQS = 2.0 ** 8      # fp8 pre-scale on w_q (SCALE-folded values are tiny)
WKS = 2.0 ** 5      # fp8 pre-scale on w_k
WVS = 2.0 ** 5      # fp8 pre-scale on w_v


@with_exitstack
def attn_core_kernel(ctx, tc, outs, ins, repeat=1):
    nc = tc.nc
    out_ap = outs["out"]
    xT, wqkT, wvT, wpT, bias, maskT = (
        ins["xT"], ins["wqkT"], ins["wvT"], ins["wpT"], ins["bias"], ins["maskT"],
    )

    consts = ctx.enter_context(tc.tile_pool(name="consts", bufs=1))
    ppool = ctx.enter_context(tc.tile_pool(name="ps", bufs=2, space="PSUM"))
    scpool = ctx.enter_context(tc.tile_pool(name="scp", bufs=2, space="PSUM"))
    ptpool = ctx.enter_context(tc.tile_pool(name="pt", bufs=9))
    rcpool = ctx.enter_context(tc.tile_pool(name="rc", bufs=4))
    aqpool = ctx.enter_context(tc.tile_pool(name="aq", bufs=2))
    outpool = ctx.enter_context(tc.tile_pool(name="ob", bufs=2))

    xT_sb = consts.tile([128, 6, 2, NK], F8)     # [.., hi/lo, n]
    wqk_sb = consts.tile([128, 6, 2, 1536], F8)
    wv_sb = consts.tile([128, 6, 2, 768], F8)
    wp_sb = consts.tile([128, 6, 768], F16)
    mask_sb = consts.tile([128, 5, 256], F16)
    bias_sb = consts.tile([128, 768], F32)
    # qk2[j]: j<6 = qT pair (heads 2j / 2j+1 in rows 0:64 / 64:128, q cols
    # 0:512); j>=6 = kT pair (key-buffer cols 0:640)
    qk2_sb = consts.tile([128, 12, NK], F16)
    vaug_sb = consts.tile([128, 5, H * 65], F16)  # [n-tile, head*(64+ones)]
    attnT_sb = consts.tile([128, 6, CHUNK], F16)  # [c-tile, n]
    ident_sb = consts.tile([128, 128], F16)

    # ---- loads ----
    xT3 = xT.rearrange("(t p) two n -> p t two n", p=128)
    wqk3 = wqkT.rearrange("(t p) two e -> p t two e", p=128)
    wv3 = wvT.rearrange("(t p) two e -> p t two e", p=128)
    wp3 = wpT.rearrange("(t p) e -> p t e", p=128)

    def loads():
        # phase A (critical for qkT q-pairs): wqk q-halves + all xT tiles,
        # interleaved over the three DMA-capable queues so the last
        # critical tile lands ~5us in (vs ~9us with whole-tile loads)
        qa = [nc.sync, nc.scalar, nc.gpsimd]
        plan = [
            (0, 'w', 0), (1, 'w', 1), (2, 'x', 0),
            (0, 'x', 1), (1, 'x', 2), (2, 'w', 2),
            (0, 'w', 3), (1, 'w', 4), (2, 'x', 3),
            (0, 'x', 4), (1, 'x', 5), (2, 'w', 5),
        ]
        for q, kind, t in plan:
            if kind == 'w':
                qa[q].dma_start(wqk_sb[:, t, :, 0:768], wqk3[:, t, :, 0:768])
            else:
                qa[q].dma_start(xT_sb[:, t], xT3[:, t])
        # phase B: k-halves (needed from pair 6, ~12us in)
        for t in range(6):
            qa[t % 3].dma_start(wqk_sb[:, t, :, 768:1536], wqk3[:, t, :, 768:1536])
        # phase C on sync only, keeping ACT/Pool queues free for evac
        # copies; ordered by first use (wv ~17us, mask ~20, wp/bias ~35+)
        for t in range(6):
            nc.sync.dma_start(wv_sb[:, t], wv3[:, t])
        for kt in range(5):
            nc.sync.dma_start(mask_sb[:, kt, :], maskT[kt])
        for t in range(6):
            nc.sync.dma_start(wp_sb[:, t, :], wp3[:, t, :])
        nc.sync.dma_start(bias_sb[:], bias[0:1, :].to_broadcast((128, 768)))
        make_identity(nc, ident_sb[:])
        # ones columns of vaug (offset 64, stride 65, 12 per key-tile)
        va = vaug_sb.rearrange("p t (h u) -> p t h u", u=65)
        for kt in range(5):
            nc.vector.memset(va[:, kt, :, 64], 1.0)

    def _copy(eng, out, in_, scale=None):
        if scale is None:
            if eng is nc.scalar:
                eng.copy(out=out, in_=in_)
            else:
                eng.tensor_copy(out=out, in_=in_)
        elif eng is nc.scalar:
            eng.activation(out=out, in_=in_,
                           func=mybir.ActivationFunctionType.Copy, scale=scale)
        else:
            eng.tensor_scalar_mul(out, in_, scale)

    evac_rr = [nc.vector, nc.scalar, nc.gpsimd]

    def qkv():
        # qkT: two 64-row head-groups per M=128 matmul; single-copy evac
        ei = 0
        for j in range(12):
            chunks = ((64, 512),) if j < 6 else ((0, 512), (512, 128))
            for c0, w in chunks:
                d0 = c0 - 64 if j < 6 else c0
                ps = ppool.tile([128, 512], F32, tag="mm")
                mi = 0
                for t in range(3):
                    for wi, xi in ((0, 0), (0, 1), (1, 0)):
                        nc.tensor.matmul(
                            ps[:, :w],
                            wqk_sb[:, 2 * t:2 * t + 2, wi, j * 128:(j + 1) * 128],
                            xT_sb[:, 2 * t:2 * t + 2, xi, c0:c0 + w],
                            start=(mi == 0), stop=(mi == 8), perf_mode=DR,
                        )
                        mi += 1
                us = 1.0 / (XS * (WQS if j < 6 else WKS))
                _copy(evac_rr[ei % 3], qk2_sb[:, j, d0:d0 + w], ps[:, :w], us)
                ei += 1
        # v -> vaug (strided per-head destination); keep ACT lighter (exp)
        vrr = [nc.vector, nc.gpsimd]
        va = vaug_sb.rearrange("p t (h u) -> p t h u", u=65)
        for nt in range(5):
            for ci, (c0, w, h0, nh) in enumerate(
                    ((0, 512, 0, 8), (512, 256, 8, 4))):
                ps = ppool.tile([128, 512], F32, tag="mm")
                mi = 0
                for t in range(3):
                    for xi, wi in ((0, 0), (0, 1), (1, 0)):
                        nc.tensor.matmul(
                            ps[:, :w],
                            xT_sb[:, 2 * t:2 * t + 2, xi, nt * 128:(nt + 1) * 128],
                            wv_sb[:, 2 * t:2 * t + 2, wi, c0:c0 + w],
                            start=(mi == 0), stop=(mi == 8), perf_mode=DR,
                        )
                        mi += 1
                _copy(vrr[(2 * nt + ci) % 2], va[:, nt, h0:h0 + nh, 0:64],
                      ps[:, :w].rearrange("p (h d) -> p h d", d=64),
                      1.0 / (XS * WVS))

    pt_tiles = {}

    def scores_kt(kt):
        # cq range actually consumed downstream; one psum tile per head
        # PAIR (odd head's operands read at partition base 64 via PE
        # quadrant tile_position)
        cq0, cq1 = (128, 256) if kt == 0 else ((0, 128) if kt == 4 else (0, 256))
        for hq in range(3):          # 4 heads per 2-bank psum tile
            sc = scpool.tile([128, 1024], F32, tag="sc")
            for j4 in range(4):
                h = 4 * hq + j4
                base = 64 * (j4 % 2)
                lhsT = qk2_sb[base:base + 64, 6 + h // 2,
                              kt * 128:kt * 128 + 128]
                rhs = qk2_sb[base:base + 64, h // 2,
                             kt * 128 - 128 + cq0:kt * 128 - 128 + cq1]
                nc.tensor.matmul(sc[:, 256 * j4 + cq0:256 * j4 + cq1], lhsT,
                                 rhs, start=True, stop=True,
                                 tile_position=(base, 0))
            pt = ptpool.tile([128, 1024], F16, tag="pt")
            sc2 = sc.rearrange("p (h q) -> p h q", h=4)
            pt2 = pt.rearrange("p (h q) -> p h q", h=4)
            nc.scalar.activation(out=pt2[:, :, cq0:cq1], in_=sc2[:, :, cq0:cq1],
                                 func=mybir.ActivationFunctionType.Exp)
            meng = nc.gpsimd if hq % 2 else nc.vector
            meng.tensor_tensor(
                pt2[:, :, cq0:cq1], pt2[:, :, cq0:cq1],
                mask_sb[:, kt:kt + 1, cq0:cq1].to_broadcast((128, 4, cq1 - cq0)),
                mybir.AluOpType.mult,
            )
            pt_tiles[(kt, hq)] = pt

    def av_r(r):
        va = vaug_sb.rearrange("p t (h u) -> p t h u", u=65)
        aq = aqpool.tile([128, 768], F16, tag="aq")
        for hg in range(3):
            av = ppool.tile([128, 260], F32, tag="av", bufs=2)
            av3 = av.rearrange("p (h u) -> p h u", u=65)
            for j in range(4):
                h = 4 * hg + j
                for ki, kt in ((0, r), (1, r + 1)):
                    col0 = 128 if ki == 0 else 0
                    pt = pt_tiles[(kt, h // 4)]
                    lhsT = pt[:, 256 * (h % 4) + col0:256 * (h % 4) + col0 + 128]
                    nc.tensor.matmul(av3[:, j, :], lhsT, va[:, kt, h, :],
                                     start=(ki == 0), stop=(ki == 1))
            rc = rcpool.tile([128, 4], F32, tag="rc")
            nc.vector.reciprocal(rc[:], av3[:, :, 64])
            nc.vector.tensor_tensor(
                aq.rearrange("p (h d) -> p h d", d=64)[:, 4 * hg:4 * hg + 4, :],
                av3[:, :, 0:64],
                rc[:, :, None].to_broadcast((128, 4, 64)),
                mybir.AluOpType.mult,
            )
            # [q, c] -> attnT [c, q] per head-group right after its norm so
            # proj's ct-ordered accumulation can start before the round ends
            tr = ppool.tile([128, 2, 128], F16, tag="av", bufs=2)
            for ci in range(2):
                ct = 2 * hg + ci
                nc.tensor.transpose(tr[:, ci, :], aq[:, 128 * ct:128 * ct + 128],
                                    ident_sb[:])
            (nc.gpsimd if hg % 2 else nc.vector).tensor_copy(
                out=attnT_sb[:, 2 * hg:2 * hg + 2, 128 * r:128 * r + 128],
                in_=tr[:])

    def proj_r(r):
        ob = outpool.tile([128, 768], F16, tag="ob")
        for cc, (c0, w) in enumerate(((0, 512), (512, 256))):
            ps = ppool.tile([128, 512], F32, tag="mm")
            for ct in range(6):
                nc.tensor.matmul(
                    ps[:, :w],
                    attnT_sb[:, ct, 128 * r:128 * r + 128],
                    wp_sb[:, ct, c0:c0 + w],
                    start=(ct == 0), stop=(ct == 5),
                )
            (nc.gpsimd if cc == 0 else nc.vector).tensor_add(
                out=ob[:, c0:c0 + w], in0=ps[:, :w], in1=bias_sb[:, c0:c0 + w])
            (nc.sync if cc == 0 else nc.scalar).dma_start(
                out_ap[128 * r:128 * r + 128, c0:c0 + w], ob[:, c0:c0 + w])

    for _rep in range(repeat):
        pt_tiles.clear()
        loads()
        qkv()
        # software-pipelined: proj of round r-1 is emitted between the
        # next scores and av so PE has dependency-free work while the
        # exp->mask chain for round r completes
        scores_kt(0)
        scores_kt(1)
        av_r(0)
        for r in range(1, 4):
            scores_kt(r + 1)
            proj_r(r - 1)
            av_r(r)
        proj_r(3)


def build_nc(repeat=1):
    nc = bacc.Bacc("TRN2", target_bir_lowering=False, debug=False)
    ins = {
        "xT": nc.dram_tensor("xT", [C, 2, NK], F8, kind="ExternalInput").ap(),
        "wqkT": nc.dram_tensor("wqkT", [C, 2, 2 * C], F8, kind="ExternalInput").ap(),
        "wvT": nc.dram_tensor("wvT", [C, 2, C], F8, kind="ExternalInput").ap(),
        "wpT": nc.dram_tensor("wpT", [C, C], F16, kind="ExternalInput").ap(),
        "bias": nc.dram_tensor("bias", [1, C], F32, kind="ExternalInput").ap(),
        "maskT": nc.dram_tensor("maskT", [5, 128, 256], F16, kind="ExternalInput").ap(),
    }
    outs = {"out": nc.dram_tensor("out", [CHUNK, C], F16, kind="ExternalOutput").ap()}
    with tile.TileContext(nc) as tc:
        attn_core_kernel(tc, outs, ins, repeat=repeat)
    nc.finalize()
    return nc


def make_core_inputs(x, w_qkv, w_proj, b_proj):
    """Build the 8 per-core input maps from full inputs."""
    x = np.asarray(x, dtype=np.float32)
    w_qkv = np.asarray(w_qkv, dtype=np.float32)
    w_proj = np.asarray(w_proj, dtype=np.float32)
    b_proj = np.asarray(b_proj, dtype=np.float32)

    f8 = mybir.dt.np(F8)

    def hilo(a):
        hi = a.astype(f8)
        lo = (a - hi.astype(np.float32)).astype(f8)
        return np.stack([hi, lo], axis=1)  # [rows, 2, cols]

    wqk = np.concatenate(
        [w_qkv[:C] * (SCALE * WQS), w_qkv[C:2 * C] * WKS], axis=0)
    wqkT = hilo(np.ascontiguousarray(wqk.T))
    wvT = hilo(np.ascontiguousarray(w_qkv[2 * C:].T) * WVS)
    wpT = np.ascontiguousarray(w_proj.T).astype(np.float16)
    bias = b_proj.reshape(1, C).astype(np.float32)

    in_maps = []
    for c in range(NCORES):
        b, s = divmod(c, 4)
        lo = s * CHUNK - HALF
        hi = s * CHUNK + CHUNK + HALF
        xs = np.zeros((NK, C), dtype=np.float32)
        s0, s1 = max(lo, 0), min(hi, N)
        xs[s0 - lo:s1 - lo] = x[b, s0:s1]
        xT = hilo(np.ascontiguousarray(xs.T) * XS)

        mask = np.zeros((5, 128, 256), dtype=np.float16)
        k = np.arange(128)[:, None]
        cq = np.arange(256)[None, :]
        band = (cq - k >= 0) & (cq - k <= 128)
        for kt in range(5):
            key_seq = s * CHUNK - HALF + 128 * kt + k
            valid = (key_seq >= 0) & (key_seq < N)
            mask[kt] = (band & valid).astype(np.float16)

        in_maps.append({
            "xT": xT, "wqkT": wqkT, "wvT": wvT, "wpT": wpT,
            "bias": bias, "maskT": mask,
        })
    return in_maps


_NC_CACHE = None


def kernel(x, w_qkv, w_proj, b_proj):
    from concourse.bass_utils import run_bass_kernel_spmd

    global _NC_CACHE
    if _NC_CACHE is None:
        _NC_CACHE = build_nc()
    in_maps = make_core_inputs(x, w_qkv, w_proj, b_proj)
    res = run_bass_kernel_spmd(_NC_CACHE, in_maps, core_ids=list(range(NCORES)))
    out = np.empty((B, N, C), dtype=np.float32)
    for c in range(NCORES):
        b, s = divmod(c, 4)
        out[b, s * CHUNK:(s + 1) * CHUNK] = res.results[c]["out"].astype(np.float32)
    return out
